# revision 23
# baseline (speedup 1.0000x reference)
"""Ragged segment mean kernel for Trainium2 (8 NeuronCores, data-parallel).

Problem: seq [64, 2048, 1024] f32, begin/end [64] i32/i64.
Output: out[i] = mean(seq[i, begin[i]:end[i], :])  -> [64, 1024] f32.

Strategy: pure data parallel over the batch, 8 samples per core, with a
host-directed "slot" architecture so each core reads only the rows its
segments actually cover:

- A slot is a 256-row (1 MiB) contiguous read of the core's seq shard
  at a runtime row offset (register-loaded from a small int32 input).
  Sample i's segment [begin, end) is covered by ceil(span/256) slots
  starting exactly at begin + 256k; slot reads always stay inside the
  sample (begin < 1024, span < 1024 => last row < 2048).
- The host bin-packs samples onto cores (a permutation of the batch)
  to equalize per-core slot counts, builds each slot's 0/1 row mask
  (segment membership), and pads cores to a common slot count S with
  zero-mask slots. S is input-dependent; compiled kernels are cached
  per S (bucketed), so unusual inputs at worst trigger a recompile,
  never a wrong result.

Per slot the masked row-sum runs on the PE: for each 128-row chunk,
acc[8, 512] += mask[128, 8].T @ chunk[128, 512], accumulated in PSUM
over all slots (the mask column routes rows to the right output row),
then scaled by 1/count and stored.

fp32 matmuls stream at 4 cycles/row on the PE (2 half-speed passes),
which would make the PE the bottleneck. Instead the data is split
exactly as x = hi + resid with hi = round_f32r(x) (ACT engine) and
resid = x - hi (DVE; exactly representable), and two float32r matmul
pairs (1 cycle/row) accumulate hi and resid into the same PSUM banks.
The result is exact to fp32 accumulation noise (~1e-7 rel).
"""

import numpy as np

import concourse.bacc as bacc
import concourse.bass as bass
import concourse.mybir as mybir
import concourse.tile as tile
from concourse.bass_utils import run_bass_kernel_spmd

B, L, D = 64, 2048, 1024
NCORES = 8
BP = B // NCORES              # 8 samples per core
NROW = BP * L                 # 16384 rows per core
U_ROWS = 128                  # rows per slot -> 0.5 MiB per dma_start
JPG = U_ROWS // 128           # 2 chunks of 128 rows per slot
FREE = 512                    # PSUM bank limit for matmul N
NMM = D // FREE               # 2 matmuls per chunk
S_BUCKET = 1                  # round slot count up to a multiple of this

_nc_cache = {}


def _build_nc(S):
    nc = bacc.Bacc("TRN2", target_bir_lowering=False)
    seq = nc.dram_tensor("seq", [NROW, D], mybir.dt.float32, kind="ExternalInput")
    maskt = nc.dram_tensor(
        "maskt", [128, S * JPG * BP], mybir.dt.float32, kind="ExternalInput"
    )
    invc = nc.dram_tensor("invc", [BP, 1], mybir.dt.float32, kind="ExternalInput")
    beg = nc.dram_tensor("beg", [S, 1], mybir.dt.int32, kind="ExternalInput")
    out = nc.dram_tensor("out", [BP, D], mybir.dt.float32, kind="ExternalOutput")

    f32 = mybir.dt.float32
    f32r = mybir.dt.float32r
    GF = U_ROWS * D // 128  # free size of one slot tile (2048)

    with tile.TileContext(nc) as tc:
        with (
            tc.tile_pool(name="const", bufs=1) as cpool,
            tc.tile_pool(name="seqp", bufs=8) as spool,
            tc.tile_pool(name="accp", bufs=1, space="PSUM") as ppool,
            tc.tile_pool(name="resp", bufs=1) as rpool,
        ):
            # bg on the SP ring (its reg_loads live there); mask/iv on the
            # ACT ring so SP reaches the first slot DMA sooner.
            bg = cpool.tile([S, 1], mybir.dt.int32, tag="bg")
            nc.sync.dma_start(out=bg[:], in_=beg[:])
            mt = cpool.tile([128, S * JPG * BP], f32)
            nc.sync.dma_start(out=mt[:], in_=maskt[:])
            # f32r-rounded mask (exact for 0/1) for the f32r matmuls
            mr = cpool.tile([128, S * JPG * BP], f32r, tag="mr")
            nc.vector.tensor_copy(out=mr[:], in_=mt[:])
            iv = cpool.tile([BP, 1], f32)
            nc.sync.dma_start(out=iv[:], in_=invc[:])
            # Pre-touch invc on DVE so the epilogue doesn't need a 2nd wait.
            iv2 = cpool.tile([BP, 1], f32, tag="iv2")
            nc.vector.tensor_copy(out=iv2[:], in_=iv[:])

            # Warmup matmul consuming only the mask tile: the self-loading
            # f32r LDWEIGHTS encoding fits a single sync wait, so the first
            # real matmul must not wait on both the mask copy and the seq
            # pipeline. This absorbs the mask dependency into the PE clock.
            warm = ppool.tile([BP, BP], f32, tag="warm")
            nc.tensor.matmul(
                out=warm[:],
                lhsT=mr[:, 0:BP],
                rhs=mr[:, 0:BP],
                start=True,
                stop=True,
            )

            acc = ppool.tile([BP, D], f32)
            for s in range(S):
                r = nc.alloc_register(mybir.EngineType.SP, f"rs{s}")
                nc.sync.reg_load(r, bg[s : s + 1, 0:1])
                off = nc.snap(r, min_val=0, max_val=NROW - U_ROWS)
                t = spool.tile([128, GF], f32)
                src = seq[bass.ds(off, U_ROWS), :].rearrange(
                    "(p j) d -> p (j d)", p=128
                )
                nc.sync.dma_start(out=t[:], in_=src)
                # exact split x = hi + resid in f32r precision
                hi = spool.tile([128, GF], f32r, tag="hi")
                nc.scalar.copy(out=hi[:], in_=t[:])
                rs = spool.tile([128, GF], f32r, tag="rs")
                nc.vector.tensor_tensor(
                    out=rs[:],
                    in0=t[:],
                    in1=hi[:].bitcast(f32),
                    op=mybir.AluOpType.subtract,
                )
                for j in range(JPG):
                    lhs = mr[:, (s * JPG + j) * BP : (s * JPG + j + 1) * BP]
                    for h in range(NMM):
                        sl = slice(j * D + h * FREE, j * D + (h + 1) * FREE)
                        nc.tensor.matmul(
                            out=acc[:, h * FREE : (h + 1) * FREE],
                            lhsT=lhs,
                            rhs=hi[:, sl],
                            start=(s == 0 and j == 0),
                            stop=False,
                        )
                        nc.tensor.matmul(
                            out=acc[:, h * FREE : (h + 1) * FREE],
                            lhsT=lhs,
                            rhs=rs[:, sl],
                            start=False,
                            stop=(s == S - 1 and j == JPG - 1),
                        )

            res = rpool.tile([BP, D], f32)
            nc.vector.tensor_scalar_mul(out=res[:], in0=acc[:], scalar1=iv2[:])
            nc.sync.dma_start(out=out[:], in_=res[:])
    nc.compile()
    return nc


def _plan(begin, end):
    """Bin-pack samples onto cores; return (perm, S, per-core slot lists).

    perm[ci*BP + i_local] = original sample index. Each slot is
    (i_local, k) meaning rows [begin + 256k, begin + 256(k+1)) of that
    local sample.
    """
    span = (end - begin).astype(np.int64)
    units = -(-span // U_ROWS)  # ceil
    order = np.argsort(-units, kind="stable")
    loads = [0] * NCORES
    members = [[] for _ in range(NCORES)]
    for si in order:
        ci = loads.index(min(loads))
        if len(members[ci]) >= BP:
            # this core is full; pick the least-loaded core with room
            ci = min(
                (c for c in range(NCORES) if len(members[c]) < BP),
                key=lambda c: loads[c],
            )
        loads[ci] += int(units[si])
        members[ci].append(int(si))
    # pad cores to exactly BP samples (all samples used exactly once)
    perm = np.array([si for ci in range(NCORES) for si in members[ci]], dtype=np.int64)
    assert len(perm) == B and len(set(perm.tolist())) == B
    S = max(loads)
    S = -(-S // S_BUCKET) * S_BUCKET
    return perm, S


def _make_in_maps(seq, begin, end, perm, S):
    in_maps = []
    p = np.arange(128)
    for ci in range(NCORES):
        samples = perm[ci * BP : (ci + 1) * BP]
        b = begin[samples].astype(np.int64)
        e = end[samples].astype(np.int64)
        span = e - b
        offs = np.zeros((S, 1), dtype=np.int32)
        mt = np.zeros((128, S * JPG * BP), dtype=np.float32)
        s = 0
        for i in range(BP):
            for k in range(int(-(-span[i] // U_ROWS))):
                offs[s, 0] = i * L + b[i] + k * U_ROWS
                for j in range(JPG):
                    # tile[p, j*D+d] holds slot row JPG*p + j
                    w = k * U_ROWS + JPG * p + j  # segment-relative row
                    mt[:, (s * JPG + j) * BP + i] = (w < span[i]).astype(np.float32)
                s += 1
        assert s <= S, (s, S)
        inv = (1.0 / span.astype(np.float64)).astype(np.float32).reshape(BP, 1)
        in_maps.append(
            {
                "seq": np.ascontiguousarray(
                    seq[samples].reshape(NROW, D), dtype=np.float32
                ),
                "maskt": mt,
                "invc": inv,
                "beg": offs,
            }
        )
    return in_maps


def _run(seq, begin, end, trace=False):
    seq = np.asarray(seq)
    begin = np.asarray(begin).astype(np.int64)
    end = np.asarray(end).astype(np.int64)
    perm, S = _plan(begin, end)
    if S not in _nc_cache:
        _nc_cache[S] = _build_nc(S)
    in_maps = _make_in_maps(seq, begin, end, perm, S)
    res = run_bass_kernel_spmd(_nc_cache[S], in_maps, list(range(NCORES)), trace=trace)
    permuted = np.concatenate(
        [res.results[ci]["out"] for ci in range(NCORES)], axis=0
    )
    out = np.empty_like(permuted)
    out[perm] = permuted
    return out, res


def kernel(seq, begin, end):
    out, _ = _run(seq, begin, end, trace=False)
    return out


# revision 24
# speedup vs baseline: 1.1005x; 1.1005x over previous
"""Ragged segment mean kernel for Trainium2 (8 NeuronCores, data-parallel).

Problem: seq [64, 2048, 1024] f32, begin/end [64] i32/i64.
Output: out[i] = mean(seq[i, begin[i]:end[i], :])  -> [64, 1024] f32.

Strategy: pure data parallel over the batch, 8 samples per core, with a
host-directed "slot" architecture so each core reads only the rows its
segments actually cover:

- A slot is a 256-row (1 MiB) contiguous read of the core's seq shard
  at a runtime row offset (register-loaded from a small int32 input).
  Sample i's segment [begin, end) is covered by ceil(span/256) slots
  starting exactly at begin + 256k; slot reads always stay inside the
  sample (begin < 1024, span < 1024 => last row < 2048).
- The host bin-packs samples onto cores (a permutation of the batch)
  to equalize per-core slot counts, builds each slot's 0/1 row mask
  (segment membership), and pads cores to a common slot count S with
  zero-mask slots. S is input-dependent; compiled kernels are cached
  per S (bucketed), so unusual inputs at worst trigger a recompile,
  never a wrong result.

Per slot the masked row-sum runs on the PE: for each 128-row chunk,
acc[8, 512] += mask[128, 8].T @ chunk[128, 512], accumulated in PSUM
over all slots (the mask column routes rows to the right output row),
then scaled by 1/count and stored.

fp32 matmuls stream at 4 cycles/row on the PE (2 half-speed passes),
which would make the PE the bottleneck. Instead the data is split
exactly as x = hi + resid with hi = round_f32r(x) (ACT engine) and
resid = x - hi (DVE; exactly representable), and two float32r matmul
pairs (1 cycle/row) accumulate hi and resid into the same PSUM banks.
The result is exact to fp32 accumulation noise (~1e-7 rel).
"""

import numpy as np

import concourse.bacc as bacc
import concourse.bass as bass
import concourse.mybir as mybir
import concourse.tile as tile
from concourse.bass_utils import run_bass_kernel_spmd

B, L, D = 64, 2048, 1024
NCORES = 8
BP = B // NCORES              # 8 samples per core
NROW = BP * L                 # 16384 rows per core
U_ROWS = 256                  # rows per slot -> 1 MiB per dma_start
JPG = U_ROWS // 128           # 2 chunks of 128 rows per slot
FREE = 512                    # PSUM bank limit for matmul N
NMM = D // FREE               # 2 matmuls per chunk
S_BUCKET = 1                  # round slot count up to a multiple of this

_nc_cache = {}


def _build_nc(S):
    nc = bacc.Bacc("TRN2", target_bir_lowering=False)
    seq = nc.dram_tensor("seq", [NROW, D], mybir.dt.float32, kind="ExternalInput")
    maskt = nc.dram_tensor(
        "maskt", [128, S * JPG * BP], mybir.dt.float32, kind="ExternalInput"
    )
    invc = nc.dram_tensor("invc", [BP, 1], mybir.dt.float32, kind="ExternalInput")
    beg = nc.dram_tensor("beg", [S, 1], mybir.dt.int32, kind="ExternalInput")
    out = nc.dram_tensor("out", [BP, D], mybir.dt.float32, kind="ExternalOutput")

    f32 = mybir.dt.float32
    f32r = mybir.dt.float32r
    GF = U_ROWS * D // 128  # free size of one slot tile (2048)

    with tile.TileContext(nc) as tc:
        with (
            tc.tile_pool(name="const", bufs=1) as cpool,
            tc.tile_pool(name="seqp", bufs=5) as spool,
            tc.tile_pool(name="accp", bufs=1, space="PSUM") as ppool,
            tc.tile_pool(name="resp", bufs=1) as rpool,
        ):
            # bg on the SP ring (its reg_loads live there); mask/iv on the
            # ACT ring so SP reaches the first slot DMA sooner.
            bg = cpool.tile([S, 1], mybir.dt.int32, tag="bg")
            nc.gpsimd.dma_start(out=bg[:], in_=beg[:])
            mt = cpool.tile([128, S * JPG * BP], f32)
            nc.sync.dma_start(out=mt[:], in_=maskt[:])
            # f32r-rounded mask (exact for 0/1) for the f32r matmuls
            mr = cpool.tile([128, S * JPG * BP], f32r, tag="mr")
            nc.vector.tensor_copy(out=mr[:], in_=mt[:])
            iv = cpool.tile([BP, 1], f32)
            nc.sync.dma_start(out=iv[:], in_=invc[:])
            # Pre-touch invc on DVE so the epilogue doesn't need a 2nd wait.
            iv2 = cpool.tile([BP, 1], f32, tag="iv2")
            nc.vector.tensor_copy(out=iv2[:], in_=iv[:])

            # Warmup matmul consuming only the mask tile: the self-loading
            # f32r LDWEIGHTS encoding fits a single sync wait, so the first
            # real matmul must not wait on both the mask copy and the seq
            # pipeline. This absorbs the mask dependency into the PE clock.
            warm = ppool.tile([BP, BP], f32, tag="warm")
            nc.tensor.matmul(
                out=warm[:],
                lhsT=mr[:, 0:BP],
                rhs=mr[:, 0:BP],
                start=True,
                stop=True,
            )

            acc = ppool.tile([BP, D], f32)
            for s in range(S):
                r = nc.alloc_register(mybir.EngineType.Pool, f"rs{s}")
                nc.gpsimd.reg_load(r, bg[s : s + 1, 0:1])
                off = nc.snap(r, min_val=0, max_val=NROW - U_ROWS)
                t = spool.tile([128, GF], f32)
                src = seq[bass.ds(off, U_ROWS), :].rearrange(
                    "(p j) d -> p (j d)", p=128
                )
                nc.gpsimd.dma_start(out=t[:], in_=src)
                # exact split x = hi + resid in f32r precision
                hi = spool.tile([128, GF], f32r, tag="hi")
                nc.scalar.copy(out=hi[:], in_=t[:])
                rs = spool.tile([128, GF], f32r, tag="rs")
                nc.vector.tensor_tensor(
                    out=rs[:],
                    in0=t[:],
                    in1=hi[:].bitcast(f32),
                    op=mybir.AluOpType.subtract,
                )
                for j in range(JPG):
                    lhs = mr[:, (s * JPG + j) * BP : (s * JPG + j + 1) * BP]
                    for h in range(NMM):
                        sl = slice(j * D + h * FREE, j * D + (h + 1) * FREE)
                        nc.tensor.matmul(
                            out=acc[:, h * FREE : (h + 1) * FREE],
                            lhsT=lhs,
                            rhs=hi[:, sl],
                            start=(s == 0 and j == 0),
                            stop=False,
                        )
                        nc.tensor.matmul(
                            out=acc[:, h * FREE : (h + 1) * FREE],
                            lhsT=lhs,
                            rhs=rs[:, sl],
                            start=False,
                            stop=(s == S - 1 and j == JPG - 1),
                        )

            res = rpool.tile([BP, D], f32)
            nc.vector.tensor_scalar_mul(out=res[:], in0=acc[:], scalar1=iv2[:])
            nc.sync.dma_start(out=out[:], in_=res[:])
    nc.compile()
    return nc


def _plan(begin, end):
    """Bin-pack samples onto cores; return (perm, S, per-core slot lists).

    perm[ci*BP + i_local] = original sample index. Each slot is
    (i_local, k) meaning rows [begin + 256k, begin + 256(k+1)) of that
    local sample.
    """
    span = (end - begin).astype(np.int64)
    units = -(-span // U_ROWS)  # ceil
    order = np.argsort(-units, kind="stable")
    loads = [0] * NCORES
    members = [[] for _ in range(NCORES)]
    for si in order:
        ci = loads.index(min(loads))
        if len(members[ci]) >= BP:
            # this core is full; pick the least-loaded core with room
            ci = min(
                (c for c in range(NCORES) if len(members[c]) < BP),
                key=lambda c: loads[c],
            )
        loads[ci] += int(units[si])
        members[ci].append(int(si))
    # pad cores to exactly BP samples (all samples used exactly once)
    perm = np.array([si for ci in range(NCORES) for si in members[ci]], dtype=np.int64)
    assert len(perm) == B and len(set(perm.tolist())) == B
    S = max(loads)
    S = -(-S // S_BUCKET) * S_BUCKET
    return perm, S


def _make_in_maps(seq, begin, end, perm, S):
    in_maps = []
    p = np.arange(128)
    for ci in range(NCORES):
        samples = perm[ci * BP : (ci + 1) * BP]
        b = begin[samples].astype(np.int64)
        e = end[samples].astype(np.int64)
        span = e - b
        offs = np.zeros((S, 1), dtype=np.int32)
        mt = np.zeros((128, S * JPG * BP), dtype=np.float32)
        s = 0
        for i in range(BP):
            for k in range(int(-(-span[i] // U_ROWS))):
                offs[s, 0] = i * L + b[i] + k * U_ROWS
                for j in range(JPG):
                    # tile[p, j*D+d] holds slot row JPG*p + j
                    w = k * U_ROWS + JPG * p + j  # segment-relative row
                    mt[:, (s * JPG + j) * BP + i] = (w < span[i]).astype(np.float32)
                s += 1
        assert s <= S, (s, S)
        inv = (1.0 / span.astype(np.float64)).astype(np.float32).reshape(BP, 1)
        in_maps.append(
            {
                "seq": np.ascontiguousarray(
                    seq[samples].reshape(NROW, D), dtype=np.float32
                ),
                "maskt": mt,
                "invc": inv,
                "beg": offs,
            }
        )
    return in_maps


def _run(seq, begin, end, trace=False):
    seq = np.asarray(seq)
    begin = np.asarray(begin).astype(np.int64)
    end = np.asarray(end).astype(np.int64)
    perm, S = _plan(begin, end)
    if S not in _nc_cache:
        _nc_cache[S] = _build_nc(S)
    in_maps = _make_in_maps(seq, begin, end, perm, S)
    res = run_bass_kernel_spmd(_nc_cache[S], in_maps, list(range(NCORES)), trace=trace)
    permuted = np.concatenate(
        [res.results[ci]["out"] for ci in range(NCORES)], axis=0
    )
    out = np.empty_like(permuted)
    out[perm] = permuted
    return out, res


def kernel(seq, begin, end):
    out, _ = _run(seq, begin, end, trace=False)
    return out
